# revision 6
# baseline (speedup 1.0000x reference)
"""AtomAttentionDecoder — 8-core Trainium2 kernel, v7.

v9 + the additive score mask/constant table (ccn + key -1e9 mask, a pure
function of atom_mask and block constants) is built on the host and staged,
removing the device-side windows(am)/negmask/addend construction.
"""

import os
import numpy as np

B, NTOK, NATOM = 2, 512, 16384
C_TOKEN, C_ATOM, C_PAIR, C_S = 384, 128, 16, 384
NQ, NK, H, NB = 32, 128, 4, 3
DH = C_ATOM // H
NW = NATOM // NQ

WSLICES = 4
KEPT_W = NW // WSLICES
HALO_W = 8
LOC_W = KEPT_W + 2 * HALO_W
KEPT_A = KEPT_W * NQ
HALO_A = HALO_W * NQ
LOC_A = LOC_W * NQ

_jitted = None


def _build_shard_fn():
    import jax
    import jax.numpy as jnp
    bf16 = jnp.bfloat16
    f32 = jnp.float32

    def _ln(x, eps=1e-5):
        mu = jnp.mean(x, -1, keepdims=True)
        var = jnp.mean((x - mu) ** 2, -1, keepdims=True)
        return (x - mu) * jax.lax.rsqrt(var + eps)

    def _lnb(x, eps=1e-5):
        # bf16 in/out, f32 reduction accumulators
        mu = jnp.mean(x, -1, keepdims=True, dtype=f32).astype(bf16)
        d = x - mu
        var = jnp.mean((d * d).astype(f32), -1, keepdims=True)
        return d * jax.lax.rsqrt(var + eps).astype(bf16)

    def shard_fn(a, ef, plm, am, onehot, addend,
                 Wa, lnq_g, lnq_b, Wout,
                 Wgcat, bgcat, Wbcat, wqkvg, bqs, wo, wt12, wto, Wc):
        # a, ef, plm, am and all W* except lnq/Wout/ccn arrive bf16
        q = onehot @ (a @ Wa)                                # bf16 [LOC_A,128]
        q = (q + ef) * am[:, None]                           # bf16
        s = jnp.pad(ef, ((0, 0), (0, C_S - C_ATOM)))
        sn = _ln(s)                                          # bf16 [LOC_A,384]

        G = jax.nn.sigmoid(sn @ Wgcat + bgcat)               # bf16 [.,1536]
        Bc = sn @ Wbcat                                      # bf16 [.,768]

        NBLK = NK // NQ  # 4

        def windows(t):
            pad = [(48, 80)] + [(0, 0)] * (t.ndim - 1)
            tp = jnp.pad(t, pad)
            blk = tp.reshape((LOC_W + NBLK, NQ) + t.shape[1:])
            w = jnp.stack([blk[j:j + LOC_W] for j in range(NBLK)], axis=1)
            return w.reshape((LOC_W, NK) + t.shape[1:])

        # pair bias, one pass (bf16 plm, f32 accumulate); Wc has a
        # trailing ones column so raw13[...,12] = sum over c
        raw13 = jnp.einsum('wqkc,ch->wqkh', plm, Wc,
                           preferred_element_type=f32)       # [.,NB*H+1] f32
        raw = raw13[..., :NB * H]
        mu_p = raw13[..., NB * H:] * (1.0 / C_PAIR)
        msq = jnp.mean((plm * plm).astype(f32), -1, keepdims=True)
        var_p = msq - mu_p * mu_p
        r_p = jax.lax.rsqrt(var_p + 1e-5)                    # f32
        braw = (raw * r_p + addend[:, None, :, :]).astype(bf16)

        x = q
        for i in range(NB):
            gA = G[:, (4 * i + 0) * 128:(4 * i + 1) * 128]
            gS = G[:, (4 * i + 1) * 128:(4 * i + 2) * 128]
            gT = G[:, (4 * i + 2) * 128:(4 * i + 3) * 128]
            gK = G[:, (4 * i + 3) * 128:(4 * i + 4) * 128]
            bA = Bc[:, (2 * i + 0) * 128:(2 * i + 1) * 128]
            bT = Bc[:, (2 * i + 1) * 128:(2 * i + 2) * 128]

            xab = gA * _lnb(x) + bA                          # bf16
            qkvg = xab @ wqkvg[i]                            # bf16 [LOC_A,512]
            qh = (qkvg[:, 0:128] + bqs[i]).reshape(LOC_W, NQ, H, DH)
            gate = jax.nn.sigmoid(qkvg[:, 384:512])          # bf16
            kvw = windows(qkvg[:, 128:384])                  # [LOC_W,NK,256]
            kw = kvw[..., 0:128].reshape(LOC_W, NK, H, DH)
            vw = kvw[..., 128:256].reshape(LOC_W, NK, H, DH)
            scores = jnp.einsum('wqhd,wkhd->wqkh', qh, kw) \
                + braw[..., i * H:(i + 1) * H]
            e = jnp.exp(scores)                              # bf16
            denom = jnp.sum(e, axis=2, dtype=f32) + 1e-30    # [LOC_W,NQ,H]
            recip = (1.0 / denom).astype(bf16)
            o = jnp.einsum('wqkh,wkhd->wqhd', e, vw)         # bf16
            o = o * recip[..., None]
            go = gate * o.reshape(LOC_A, C_ATOM)
            x = x + gS * jnp.einsum('ac,cd->ad', go, wo[i])

            xtb = gT * _lnb(x) + bT                          # bf16
            h12 = xtb @ wt12[i]                              # bf16 [LOC_A,512]
            hsw = jax.nn.silu(h12[:, :256]) * h12[:, 256:]   # bf16
            x = x + gK * jnp.einsum('ac,cd->ad', hsw, wto[i])

        x = (x * am[:, None]).astype(f32)
        r = (_ln(x) * lnq_g + lnq_b) @ Wout                  # f32 [LOC_A,3]
        return r[HALO_A:HALO_A + KEPT_A]

    return jax.pmap(shard_fn, devices=jax.devices()[:8])


def _pad_slice(arr, lo, hi):
    n = arr.shape[0]
    lo_pad = max(0, -lo)
    hi_pad = max(0, hi - n)
    core = arr[max(lo, 0):min(hi, n)]
    if lo_pad or hi_pad:
        pad = [(lo_pad, hi_pad)] + [(0, 0)] * (arr.ndim - 1)
        core = np.pad(core, pad)
    return core


def _prep_weights(inputs):
    """Host-side weight consolidation (numpy); big ones cast to bf16."""
    import ml_dtypes
    b16 = ml_dtypes.bfloat16
    g = lambda k: np.asarray(inputs[k], np.float32)
    inv = 1.0 / np.sqrt(DH)
    Wgcat = np.concatenate([np.concatenate(
        [g('ag_w')[i], g('sk_w')[i], g('tg_w')[i], g('tk_w')[i]], axis=1)
        for i in range(NB)], axis=1).astype(b16)            # [384, 12*128]
    bgcat = np.concatenate([np.concatenate(
        [g('ag_b')[i], g('sk_b')[i], g('tg_b')[i], g('tk_b')[i]])
        for i in range(NB)]).astype(b16)                    # [12*128]
    Wbcat = np.concatenate([np.concatenate(
        [g('ab_w')[i], g('tb_w')[i]], axis=1)
        for i in range(NB)], axis=1).astype(b16)            # [384, 6*128]
    wqkvg = np.stack([np.concatenate(
        [g('wq')[i] * inv, g('wk')[i], g('wv')[i], g('wg')[i]], axis=1)
        for i in range(NB)]).astype(b16)                    # [NB, 128, 512]
    bqs = np.stack([g('bq')[i] * inv for i in range(NB)]).astype(b16)
    wt12 = np.stack([np.concatenate(
        [g('wt1')[i], g('wt2')[i]], axis=1) for i in range(NB)]).astype(b16)
    cen = np.eye(C_PAIR, dtype=np.float32) - 1.0 / C_PAIR
    Wc = np.concatenate([cen @ (g('pg')[i][:, None] * g('wpb')[i])
                         for i in range(NB)] + [np.ones((C_PAIR, 1), np.float32)],
                        axis=1).astype(b16)                 # [16, NB*H+1]
    return [g('Wa').astype(b16), g('lnq_g'), g('lnq_b'), g('Wout'),
            Wgcat, bgcat, Wbcat, wqkvg, bqs, g('wo').astype(b16), wt12,
            g('wto').astype(b16), Wc]


def stage_args(inputs):
    """Build stacked [8, ...] pmap args from full inputs (host side)."""
    import ml_dtypes
    b16 = ml_dtypes.bfloat16
    weights = _prep_weights(inputs)
    a = np.asarray(inputs['a'], np.float32).astype(b16)
    ef = np.asarray(inputs['extra_feats'], np.float32).astype(b16)
    plm = np.asarray(inputs['p_lm'], np.float32).astype(b16)
    am = np.asarray(inputs['atom_mask'], np.float32).astype(b16)
    idx = np.asarray(inputs['atom_to_token_idx'], np.int32)

    g = lambda k: np.asarray(inputs[k], np.float32)
    ccn_host = np.concatenate([g('pb')[i] @ g('wpb')[i] for i in range(NB)])
    sa, sef, splm, sam, soh, sadd = [], [], [], [], [], []
    for c in range(8):
        b, ws = divmod(c, WSLICES)
        a0 = ws * KEPT_A - HALO_A
        a1 = ws * KEPT_A + KEPT_A + HALO_A
        w0 = ws * KEPT_W - HALO_W
        w1 = ws * KEPT_W + KEPT_W + HALO_W
        sa.append(a[b])
        sef.append(_pad_slice(ef[b], a0, a1))
        splm.append(_pad_slice(plm[b], w0, w1))
        sam.append(_pad_slice(am[b], a0, a1))
        sh_idx = np.clip(_pad_slice(idx[b], a0, a1), 0, NTOK - 1)
        oh = np.zeros((LOC_A, NTOK), np.float32)
        oh[np.arange(LOC_A), sh_idx] = 1.0
        soh.append(oh.astype(b16))
        # host-built additive table: ccn + (-1e9 where key invalid)
        amp = np.pad(sam[-1].astype(np.float32), (48, 80))
        km = np.lib.stride_tricks.sliding_window_view(amp, NK)[::NQ][:LOC_W]
        add = np.where(km[:, :, None] > 0, 0.0, -1e9) + ccn_host[None, None, :]
        sadd.append(add.astype(np.float32))
    args = [np.stack(sa), np.stack(sef), np.stack(splm), np.stack(sam),
            np.stack(soh), np.stack(sadd)]
    args += [np.broadcast_to(w, (8,) + w.shape).copy() for w in weights]
    return args


def kernel(**inputs) -> np.ndarray:
    global _jitted
    if _jitted is None:
        _jitted = _build_shard_fn()
    f = _jitted

    outs = np.asarray(f(*stage_args(inputs)))   # [8, KEPT_A, 3]
    full = np.empty((B, NATOM, 3), np.float32)
    for c in range(8):
        b, ws = divmod(c, WSLICES)
        full[b, ws * KEPT_A:(ws + 1) * KEPT_A] = outs[c]
    return full
